# revision 24
# baseline (speedup 1.0000x reference)
"""AdaptiveSpectrumLayer Trainium2 kernel — 8-core pure data parallel, v2.

Pipeline per core (B_local=8 batches, COLS=1024 columns = (b,f)):
  rfft (fp32r DFT matmuls, fp32 PSUM accumulate)
  -> mag/s/c features in bf16 (DVE 2x ops; no trig: s=im/mag, c=re/mag)
  -> per-freq 4->32->2 relu MLP (block-diag bf16 matmuls; bias folded
     into matmul via constant ones-row in ff; relus split Act/DVE/Pool)
  -> reduce into paired [128x1024] PSUM tiles (2 freq groups/tile),
     full-width relu/sigmoid, DMA extraction of m/ph
  -> gate collapsed to three 257x257 matmuls (bias via ones-row)
     -> swish -> sigmoid weights (bf16)
  -> spectrum blend in bf16 (DVE 2x) -> irfft (bf16 DFT matmuls),
     output DMA'd straight from PSUM.
"""
import sys
import numpy as np

sys.path.insert(0, "/opt/trn_rl_repo")

import ml_dtypes
from contextlib import ExitStack

import concourse.bass as bass
import concourse.tile as tile
from concourse import mybir
from concourse import bacc
from concourse.bass_utils import run_bass_kernel_spmd


def _ensure_ntff_hook():
    """The agent image's antenv lacks axon_hooks; inject a stub and register
    the ctypes NTFF profiling hook so trace=True works. Safe no-op if parts
    are missing."""
    try:
        import antenv.axon_hooks  # noqa: F401
        return
    except ImportError:
        pass
    try:
        import types
        import antenv
        mod = types.ModuleType("antenv.axon_hooks")
        _state = {"hook": None}
        mod.set_axon_ntff_profile_hook = lambda h: _state.__setitem__("hook", h)
        mod.get_axon_ntff_profile_hook = lambda: _state["hook"]
        sys.modules["antenv.axon_hooks"] = mod
        antenv.axon_hooks = mod
        from trn_agent_boot.trn_boot import _ntff_profile_via_ctypes
        so = "/opt/axon/libaxon_pjrt.so"
        import os
        if os.path.exists(so):
            mod.set_axon_ntff_profile_hook(_ntff_profile_via_ctypes(so))
    except Exception:
        pass


_ensure_ntff_hook()

# ---- problem constants (hardcoded; kernel.py must be self-contained) ----
B, H, F, HID = 64, 512, 128, 32
FS = 100.0
NF = H // 2 + 1          # 257
NFP = 288                # padded freq count: 9 groups of 32 = 3 chunks of 96
NG = 9                   # freq groups (32 each)
NCH = 3                  # freq chunks (96 each)
CPW = 96                 # chunk width
NCORE = 8
BL = B // NCORE          # 8
COLS = BL * F            # 1024
NC2 = 2                  # 512-wide N chunks per 1024 cols
NPAIR = 5                # reduce pairs: (0,1)(2,3)(4,5)(6,7)(8)
EPS = 1e-30

f32 = mybir.dt.float32
f32r = mybir.dt.float32r
bf16 = mybir.dt.bfloat16
AF = mybir.ActivationFunctionType
ALU = mybir.AluOpType


# =========================================================================
# Host-side weight preparation
# =========================================================================
def build_host_weights(Wp, bp, Wg, bg, Wm, bm, Wph, bph):
    freqs = np.fft.rfftfreq(H, 1.0 / FS)[:NF].astype(np.float32)

    n_idx = np.arange(NFP)
    t_idx = np.arange(H)
    valid = (n_idx < NF).astype(np.float32)
    theta = 2.0 * np.pi * np.outer(t_idx, n_idx) / H  # (512, 288)
    inv_sqrt_h = 1.0 / np.sqrt(H)

    RC = (np.cos(theta) * inv_sqrt_h * valid[None, :]).astype(np.float32)
    RS = (-np.sin(theta) * inv_sqrt_h * valid[None, :]).astype(np.float32)
    w_n = np.where((n_idx == 0) | (n_idx == 256), 1.0, 2.0) * valid
    IC = (np.cos(theta) * inv_sqrt_h * w_n[None, :]).astype(np.float32)
    IS = (-np.sin(theta) * inv_sqrt_h * w_n[None, :]).astype(np.float32)

    WpP = np.zeros((NFP, 4, HID), np.float32); WpP[:NF] = Wp
    bpP = np.zeros((NFP, HID), np.float32);    bpP[:NF] = bp
    WmP = np.zeros((NFP, HID), np.float32);    WmP[:NF] = Wm
    bmP = np.zeros((NFP,), np.float32);        bmP[:NF] = bm
    WphP = np.zeros((NFP, HID), np.float32);   WphP[:NF] = Wph
    bphP = np.zeros((NFP,), np.float32);       bphP[:NF] = bph
    fP = np.zeros((NFP,), np.float32);         fP[:NF] = freqs

    D = fP[:, None] * WpP[:, 3, :] + bpP  # (288, 32)

    WgR = Wg.reshape(NF, HID, NF)
    G = np.zeros((3, NFP, NFP), np.float32)
    for f in range(3):
        G[f, :NF, :NF] = np.einsum("nh,nhj->nj", Wp[:, f, :], WgR)
    gconst = np.zeros((NFP,), np.float32)
    gconst[:NF] = np.einsum("nh,nhj->j", D[:NF], WgR) + bg

    # ---- device layouts ----
    # w_rfft (128, 4, 2, 3, 96) fp32: [tp][kt][ri][ch][fc]
    w_rfft = np.zeros((128, 4, 2, NCH, CPW), np.float32)
    RCr = RC.reshape(4, 128, NCH, CPW)  # [kt][tp][ch][fc]
    RSr = RS.reshape(4, 128, NCH, CPW)
    w_rfft[:, :, 0] = RCr.transpose(1, 0, 2, 3)
    w_rfft[:, :, 1] = RSr.transpose(1, 0, 2, 3)

    # w_proj (128, 9, 8, 128) bf16; row 96 carries D (ff row 96 == 1)
    w_proj = np.zeros((128, NG, 8, 128), np.float32)
    ii = np.arange(32)
    for g in range(NG):
        n = 32 * g + ii  # (32,)
        for f in range(3):
            feat = WpP[n, f, :]  # (32, 32) [i, h]
            for j in range(8):
                blk = feat[:, 4 * j:4 * j + 4]  # (32 i, 4 hh)
                for hh in range(4):
                    w_proj[32 * f + ii, g, j, 4 * ii + hh] = blk[:, hh]
        for j in range(8):
            for hh in range(4):
                w_proj[96, g, j, 4 * ii + hh] = D[n, 4 * j + hh]

    # w_red (128, 9, 8, 64): [4i+hh][g][j][col]
    w_red = np.zeros((128, NG, 8, 64), np.float32)
    for g in range(NG):
        n = 32 * g + ii
        for j in range(8):
            for hh in range(4):
                w_red[4 * ii + hh, g, j, ii] = WmP[n, 4 * j + hh]
                w_red[4 * ii + hh, g, j, 32 + ii] = WphP[n, 4 * j + hh]

    # mp_bias2 (128, 5): pair p rows = [bm g | bph g | bm g' | bph g']
    mp_bias2 = np.zeros((128, NPAIR), np.float32)
    for p in range(NPAIR):
        g, g2 = 2 * p, 2 * p + 1
        mp_bias2[0:32, p] = bmP[32 * g + ii]
        mp_bias2[32:64, p] = bphP[32 * g + ii]
        if g2 < NG:
            mp_bias2[64:96, p] = bmP[32 * g2 + ii]
            mp_bias2[96:128, p] = bphP[32 * g2 + ii]

    # w_gate (128, 9, 3, 96): [32f+i][g][jt][jc]; row 96 of g=0 carries gconst
    w_gate = np.zeros((128, NG, NCH, CPW), np.float32)
    for g in range(NG):
        n = 32 * g + ii
        for f in range(3):
            Gr = G[f][n].reshape(32, NCH, CPW)  # [i][jt][jc]
            w_gate[32 * f + ii, g] = Gr
    w_gate[96, 0] = gconst.reshape(NCH, CPW)

    # w_irfft (96, 2, 3, 4, 128): [p][ri][ch][mt][tc]
    w_irfft = np.zeros((CPW, 2, NCH, 4, 128), np.float32)
    ICr = IC.reshape(4, 128, NCH, CPW)  # [mt][tc][ch][p]
    ISr = IS.reshape(4, 128, NCH, CPW)
    w_irfft[:, 0] = ICr.transpose(3, 2, 0, 1)
    w_irfft[:, 1] = ISr.transpose(3, 2, 0, 1)

    tobf = lambda a: a.astype(ml_dtypes.bfloat16)
    return dict(
        w_rfft=w_rfft,
        w_proj=tobf(w_proj),
        w_red=tobf(w_red),
        mp_bias2=mp_bias2,
        w_gate=tobf(w_gate),
        w_irfft=tobf(w_irfft),
        ones=np.ones((1, NG, COLS), np.float32).astype(ml_dtypes.bfloat16),
    )


# =========================================================================
# Device kernel builder
# =========================================================================
def build_kernel():
    nc = bacc.Bacc()

    x_d = nc.declare_dram_parameter("x", [BL, H, F], f32, isOutput=False)
    w_rfft_d = nc.declare_dram_parameter("w_rfft", [128, 4, 2, NCH, CPW], f32, isOutput=False)
    w_proj_d = nc.declare_dram_parameter("w_proj", [128, NG, 8, 128], bf16, isOutput=False)
    w_red_d = nc.declare_dram_parameter("w_red", [128, NG, 8, 64], bf16, isOutput=False)
    mp_bias2_d = nc.declare_dram_parameter("mp_bias2", [128, NPAIR], f32, isOutput=False)
    w_gate_d = nc.declare_dram_parameter("w_gate", [128, NG, NCH, CPW], bf16, isOutput=False)
    w_irfft_d = nc.declare_dram_parameter("w_irfft", [CPW, 2, NCH, 4, 128], bf16, isOutput=False)
    ones_d = nc.declare_dram_parameter("ones", [1, NG, COLS], bf16, isOutput=False)
    out_d = nc.declare_dram_parameter("out", [BL, H, F], f32, isOutput=True)

    TWO_PI = float(2.0 * np.pi)
    PI = float(np.pi)

    with tile.TileContext(nc) as tc, ExitStack() as ctx:
        consts = ctx.enter_context(tc.tile_pool(name="consts", bufs=1))
        scratch = ctx.enter_context(tc.tile_pool(name="scratch", bufs=1))
        xr_pool = ctx.enter_context(tc.tile_pool(name="xr", bufs=12))
        rs_pool = ctx.enter_context(tc.tile_pool(name="rs", bufs=2))
        gt_pool = ctx.enter_context(tc.tile_pool(name="gt", bufs=2))
        bl_pool = ctx.enter_context(tc.tile_pool(name="bl", bufs=1))

        ps_proj = ctx.enter_context(tc.tile_pool(name="ps_proj", bufs=3, space="PSUM"))
        ps_red = ctx.enter_context(tc.tile_pool(name="ps_red", bufs=1, space="PSUM"))
        ps_misc = ctx.enter_context(tc.tile_pool(name="ps_misc", bufs=3, space="PSUM"))

        # ---- persistent SBUF tensors ----
        x_sb = consts.tile([128, 4, BL, F], f32r, tag="x_in")
        w_rfft_sb = consts.tile([128, 4, 2, NCH, CPW], f32r, tag="w_rfft")
        w_proj_sb = consts.tile([128, NG, 8, 128], bf16, tag="w_proj")
        w_red_sb = consts.tile([128, NG, 8, 64], bf16, tag="w_red")
        mp_bias2_sb = consts.tile([128, NPAIR], f32, tag="mp_bias2")
        w_gate_sb = consts.tile([128, NG, NCH, CPW], bf16, tag="w_gate")
        w_irfft_sb = consts.tile([CPW, 2, NCH, 4, 128], bf16, tag="w_irfft")

        re_sb = consts.tile([CPW, NCH, COLS], bf16, tag="re")
        im_sb = consts.tile([CPW, NCH, COLS], bf16, tag="im")
        ff_sb = consts.tile([97, NG, COLS], bf16, tag="ff")
        m_sb = consts.tile([CPW, NCH, COLS], bf16, tag="m_t")
        ph_sb = consts.tile([CPW, NCH, COLS], f32, tag="ph_t")
        w_sb = consts.tile([CPW, NCH, COLS], bf16, tag="w_t")
        sin_sb = consts.tile([CPW, NCH, COLS], bf16, tag="sin_t")
        cos_sb = consts.tile([CPW, NCH, COLS], bf16, tag="cos_t")

        # ---- load weights + input ----
        x_r = x_d[:].bitcast(f32r).rearrange("b (kt p) f -> kt p b f", p=128)
        w_rfft_r = w_rfft_d[:].bitcast(f32r)
        nc.sync.dma_start(out=w_rfft_sb[:, :, :, 0], in_=w_rfft_r[:, :, :, 0])
        for kt in range(4):
            nc.sync.dma_start(out=x_sb[:, kt], in_=x_r[kt])
        nc.sync.dma_start(out=w_rfft_sb[:, :, :, 1], in_=w_rfft_r[:, :, :, 1])
        nc.sync.dma_start(out=w_rfft_sb[:, :, :, 2], in_=w_rfft_r[:, :, :, 2])
        nc.gpsimd.dma_start(out=w_proj_sb, in_=w_proj_d[:])
        # ff row 96 := 1.0 (bias path for proj/gate; rows 0..95 DMA'd later)
        nc.gpsimd.dma_start(out=ff_sb[96:97, :, :], in_=ones_d[:])
        nc.gpsimd.dma_start(out=w_red_sb, in_=w_red_d[:])
        nc.gpsimd.dma_start(out=mp_bias2_sb, in_=mp_bias2_d[:])
        nc.gpsimd.dma_start(out=w_gate_sb, in_=w_gate_d[:])
        nc.gpsimd.dma_start(out=w_irfft_sb, in_=w_irfft_d[:])

        def const_col(value, tag):
            t = consts.tile([128, 1], f32, tag=tag)
            nc.vector.memset(t, value)
            return t

        eps_c = const_col(EPS, "c_eps")
        pi_c = const_col(PI, "c_pi")

        # ================= rfft + features, per chunk =================
        def rfft_chunk(ch):
            for ri in range(2):
                dst = re_sb if ri == 0 else im_sb
                pts = [ps_misc.tile([128, 512], f32, tag="ps_misc",
                                    name="ps_misc")[:CPW] for _ in range(NC2)]
                for kt in range(4):
                    for nck in range(NC2):
                        nc.tensor.matmul(
                            out=pts[nck],
                            lhsT=w_rfft_sb[:, kt, ri, ch, :],
                            rhs=x_sb[:, kt, 4 * nck:4 * (nck + 1), :],
                            start=(kt == 0),
                            stop=(kt == 3),
                        )
                for nck in range(NC2):
                    # PSUM fp32 -> SBUF bf16
                    nc.vector.tensor_copy(
                        out=dst[:, ch, 512 * nck:512 * (nck + 1)], in_=pts[nck]
                    )

        def features_chunk(ch):
            re_c = re_sb[:, ch, :]
            im_c = im_sb[:, ch, :]
            msq = scratch.tile([CPW, COLS], bf16, tag="msq")
            t2 = scratch.tile([CPW, COLS], bf16, tag="tmpb")
            nc.vector.tensor_mul(out=msq, in0=re_c, in1=re_c)
            nc.vector.tensor_mul(out=t2, in0=im_c, in1=im_c)
            nc.vector.tensor_add(out=msq, in0=msq, in1=t2)
            magf = scratch.tile([CPW, COLS], f32, tag="magf")
            nc.scalar.activation(out=magf, in_=msq, func=AF.Sqrt,
                                 bias=eps_c[:CPW], scale=1.0)
            rr = scratch.tile([CPW, COLS], f32, tag="rr")
            nc.vector.reciprocal_approx_fast(out=rr, in_=magf)
            magb = scratch.tile([CPW, COLS], bf16, tag="magb")
            nc.gpsimd.tensor_copy(out=magb, in_=magf)
            sbf = scratch.tile([CPW, COLS], bf16, tag="sbf")
            cbf = scratch.tile([CPW, COLS], bf16, tag="cbf")
            nc.vector.tensor_mul(out=sbf, in0=im_c, in1=rr)
            nc.vector.tensor_mul(out=cbf, in0=re_c, in1=rr)
            # interleave into FF via SBUF->SBUF DMA (cross-partition)
            for q in range(3):  # groups 3*ch + q, rows 32q..32q+32
                g = 3 * ch + q
                for f, srct in enumerate((magb, sbf, cbf)):
                    nc.gpsimd.dma_start(
                        out=ff_sb[32 * f:32 * f + 32, g, :],
                        in_=srct[32 * q:32 * q + 32, :])

        for ch in range(NCH):
            rfft_chunk(ch)
            features_chunk(ch)

        # ================= per-freq MLP =================
        # relu engine pattern per (j, nck): A=scalar, D=vector
        RELU_PAT = "ADADADAD" "ADADADAA"

        pair_tiles = {}

        def mlp_group(g):
            p = g // 2
            if g % 2 == 0:
                pair_tiles[p] = ps_red.tile([128, 1024], f32, tag="ps_red",
                                            name="ps_red")
            pair_pt = pair_tiles[p]
            r0 = 64 * (g % 2)
            xrs = []
            k = 0
            for j in range(8):
                xr2 = []
                for nck in range(NC2):
                    proj_pt = ps_proj.tile([128, 512], f32, tag="ps_proj",
                                           name="ps_proj")
                    nc.tensor.matmul(
                        out=proj_pt,
                        lhsT=w_proj_sb[:97, g, j, :],
                        rhs=ff_sb[:, g, 512 * nck:512 * (nck + 1)],
                        start=True, stop=True,
                    )
                    xr = xr_pool.tile([128, 512], bf16, tag="xr")
                    eng = RELU_PAT[k]
                    k += 1
                    if eng == "A":
                        nc.scalar.activation(out=xr, in_=proj_pt, func=AF.Relu,
                                             bias=0.0, scale=1.0)
                    elif eng == "D":
                        nc.vector.tensor_scalar(
                            out=xr, in0=proj_pt, scalar1=0.0, scalar2=None,
                            op0=ALU.max)
                    else:
                        nc.gpsimd.tensor_scalar(
                            out=xr, in0=proj_pt, scalar1=0.0, scalar2=None,
                            op0=ALU.max)
                    xr2.append(xr)
                xrs.append(xr2)
            for j in range(8):
                for nck in range(NC2):
                    nc.tensor.matmul(
                        out=pair_pt[r0:r0 + 64, 512 * nck:512 * (nck + 1)],
                        lhsT=w_red_sb[:, g, j, :],
                        rhs=xrs[j][nck],
                        start=(j == 0), stop=(j == 7),
                    )

        def pair_acts(p):
            pair_pt = pair_tiles[p]
            rows = 128 if 2 * p + 1 < NG else 64
            R = rs_pool.tile([128, 1024], bf16, tag="mpR")
            S = rs_pool.tile([128, 1024], f32, tag="mpS")
            nc.scalar.activation(out=R[:rows], in_=pair_pt[:rows],
                                 func=AF.Relu,
                                 bias=mp_bias2_sb[:rows, p:p + 1], scale=1.0)
            nc.scalar.activation(out=S[:rows], in_=pair_pt[:rows],
                                 func=AF.Sigmoid,
                                 bias=mp_bias2_sb[:rows, p:p + 1], scale=1.0)
            for gg in (2 * p, 2 * p + 1):
                if gg >= NG:
                    continue
                r0 = 64 * (gg % 2)
                ch, p0 = gg // 3, 32 * (gg % 3)
                nc.gpsimd.dma_start(out=m_sb[p0:p0 + 32, ch, :],
                                    in_=R[r0:r0 + 32, :])
                nc.gpsimd.dma_start(out=ph_sb[p0:p0 + 32, ch, :],
                                    in_=S[r0 + 32:r0 + 64, :])

        def trig_chunk(ch):
            ph_c = ph_sb[:, ch, :]
            shalf = bl_pool.tile([CPW, COLS], f32, tag="shalf")
            # Sin spline valid on [-pi, pi]:
            # sin(2pi u) = sin(pi - 2pi u); cos(2pi u) = 1 - 2 sin^2(pi u)
            nc.scalar.activation(out=shalf, in_=ph_c, func=AF.Sin,
                                 bias=0.0, scale=PI)
            nc.scalar.activation(out=sin_sb[:, ch, :], in_=ph_c, func=AF.Sin,
                                 bias=pi_c[:CPW], scale=-TWO_PI)
            sh2 = bl_pool.tile([CPW, COLS], bf16, tag="sh2")
            nc.vector.tensor_mul(out=sh2, in0=shalf, in1=shalf)
            nc.vector.tensor_scalar(out=cos_sb[:, ch, :], in0=sh2,
                                    scalar1=-2.0, scalar2=1.0,
                                    op0=ALU.mult, op1=ALU.add)

        def gate_jt(jt):
            gps = [ps_misc.tile([128, 512], f32, tag="ps_misc",
                                name="ps_misc")[:CPW] for _ in range(NC2)]
            for g in range(NG):
                for nck in range(NC2):
                    nc.tensor.matmul(
                        out=gps[nck],
                        lhsT=w_gate_sb[:97, g, jt, :],
                        rhs=ff_sb[:, g, 512 * nck:512 * (nck + 1)],
                        start=(g == 0), stop=(g == NG - 1),
                    )
            gt = gt_pool.tile([CPW, COLS], bf16, tag="gt")
            for nck in range(NC2):
                cs = slice(512 * nck, 512 * (nck + 1))
                sg = gt_pool.tile([CPW, 512], bf16, tag="sg")
                nc.scalar.activation(out=sg, in_=gps[nck], func=AF.Sigmoid,
                                     bias=0.0, scale=1.0)
                # gt = (gp + 0) * sigmoid(gp)  (swish)
                nc.vector.scalar_tensor_tensor(
                    out=gt[:, cs], in0=gps[nck], scalar=0.0, in1=sg,
                    op0=ALU.add, op1=ALU.mult)
            nc.scalar.activation(out=w_sb[:, jt, :], in_=gt, func=AF.Sigmoid,
                                 bias=0.0, scale=1.0)

        def blend_chunk(ch):
            m_c = m_sb[:, ch, :]
            w_c = w_sb[:, ch, :]
            u = bl_pool.tile([CPW, COLS], bf16, tag="u_t")
            wm = bl_pool.tile([CPW, COLS], bf16, tag="wm_t")
            nc.vector.tensor_scalar(out=u, in0=w_c, scalar1=-1.0, scalar2=1.0,
                                    op0=ALU.mult, op1=ALU.add)
            nc.vector.tensor_mul(out=wm, in0=w_c, in1=m_c)
            for trig, dst in ((cos_sb, re_sb), (sin_sb, im_sb)):
                a = bl_pool.tile([CPW, COLS], bf16, tag="a_t")
                b = bl_pool.tile([CPW, COLS], bf16, tag="b_t")
                nc.vector.tensor_mul(out=a, in0=wm, in1=trig[:, ch, :])
                nc.vector.tensor_mul(out=b, in0=u, in1=dst[:, ch, :])
                nc.vector.tensor_add(out=dst[:, ch, :], in0=a, in1=b)

        # MLP with pair acts / trig / gate / blend interleaved
        for g in range(NG):
            mlp_group(g)
            if g % 2 == 1 or g == NG - 1:
                pair_acts(g // 2)
            if g == 4:
                gate_jt(0)
            if g == 5:
                trig_chunk(0)
                trig_chunk(1)
                blend_chunk(0)
            if g == 6:
                gate_jt(1)
                blend_chunk(1)
            if g == 7:
                gate_jt(2)
        trig_chunk(2)
        blend_chunk(2)

        # ================= irfft =================
        out_sb = consts.tile([128, 4, BL, F], f32, tag="out_sb")
        out_r = out_d[:].rearrange("b (mt p) f -> mt p b f", p=128)
        for mt in range(4):
            for nck in range(NC2):
                pt = ps_misc.tile([128, 512], f32, tag="ps_misc",
                                  name="ps_misc")
                k = 0
                for ch in range(NCH):
                    for ri, src in enumerate((re_sb, im_sb)):
                        nc.tensor.matmul(
                            out=pt,
                            lhsT=w_irfft_sb[:, ri, ch, mt, :],
                            rhs=src[:, ch, 512 * nck:512 * (nck + 1)],
                            start=(k == 0), stop=(k == 5),
                        )
                        k += 1
                nc.scalar.copy(out=out_sb[:, mt, 4 * nck:4 * (nck + 1), :],
                               in_=pt)
                nc.gpsimd.dma_start(
                    out=out_r[mt][:, 4 * nck:4 * (nck + 1), :],
                    in_=out_sb[:, mt, 4 * nck:4 * (nck + 1), :])

    nc.finalize()
    return nc


_CACHE = {}


def _get_nc():
    if "nc" not in _CACHE:
        _CACHE["nc"] = build_kernel()
    return _CACHE["nc"]


def kernel(x, Wp, bp, Wg, bg, Wm, bm, Wph, bph, _trace=False):
    x = np.ascontiguousarray(np.asarray(x, dtype=np.float32))
    hw = build_host_weights(
        np.asarray(Wp, np.float32), np.asarray(bp, np.float32),
        np.asarray(Wg, np.float32), np.asarray(bg, np.float32),
        np.asarray(Wm, np.float32), np.asarray(bm, np.float32),
        np.asarray(Wph, np.float32), np.asarray(bph, np.float32),
    )
    nc = _get_nc()
    in_maps = []
    for i in range(NCORE):
        m = {"x": np.ascontiguousarray(x[i * BL:(i + 1) * BL])}
        m.update(hw)
        in_maps.append(m)
    res = run_bass_kernel_spmd(nc, in_maps, core_ids=list(range(NCORE)),
                               trace=_trace)
    out = np.concatenate([np.asarray(r["out"]) for r in res.results], axis=0)
    if _trace:
        _CACHE["last_exec_time_ns"] = res.exec_time_ns
        _CACHE["last_results"] = res
    return out.astype(np.float32)
